# revision 20
# baseline (speedup 1.0000x reference)
"""FFT-Conv2d as direct valid cross-correlation on Trainium2 (Bass/Tile).

Math: the reference's rfft2/einsum/irfft2 pipeline is exactly a *valid*
2-D cross-correlation plus bias:

    out[b, d, i, j] = sum_{c,u,v} signal[b, c, i+u, j+v] * weight[d, c, u, v]
                      + bias[d]

with signal [16, 32, 256, 256], weight [32, 32, 31, 31] -> out [16, 32, 226, 226].

Device strategy (data-parallel, 2 batch images per NeuronCore x 8 cores):
  - Contraction dim (PE partition axis, K=128) packs 4 kernel rows x 32
    input channels.  The signal is replicated into SBUF 4x with row shifts
    of 0..3 so that one SBUF row slice provides all 128 contraction rows.
  - Output dim (PE partition axis of PSUM, M=128) packs 4 kernel-column
    subshifts s=0..3 x 32 output channels.  A column block vb covers
    kernel columns 4*vb+s; the s-shift is resolved after accumulation by
    a shifted 4-way add across PSUM partition blocks.
  - Per output-row-pair: 8 row-groups x 8 col-blocks = 64 matmuls of
    [128 x 128] @ [128 x (2*229)] accumulated in one PSUM bank, then a
    3-op vector epilogue (2 shifted adds + fused add+bias) and a DMA out.

Kernel weights/columns beyond 31 are zero-padded on the host; the signal
is zero-padded by one row/column in SBUF so the padded taps multiply
zeros (never uninitialized memory).

Dtype strategy (FFTCONV_DT=mix default): the PE's moving-operand stream
rate is dtype-dependent (measured: e4m3 2.4 GHz, f16 1.85, bf16 1.55,
e3m4 1.88 — only e4m3/e5m2 get the fast 1-byte path).  e4m3 alone fails
the 2e-2 gate (4.1e-2), so the contraction is alpha-split by kernel-row
group: groups g<4 (rows 0..15) stream the f16 signal replica, groups
g>=4 (rows 16..30) stream an e4m3 replica of 2*signal with the matching
stationary f16 weights pre-halved.  CPU-exact rel err 1.78e-2; HW fp8
numerics match the CPU model to 4 digits (subnormals honored).
"""

import os
import sys

import numpy as np

for _p in ("/opt/trn_rl_repo",):
    if _p not in sys.path and os.path.isdir(_p):
        sys.path.insert(0, _p)

import concourse.bacc as bacc
import concourse.mybir as mybir
import concourse.tile as tile
from concourse.bass_utils import run_bass_kernel_spmd

# Problem constants (hardcoded per harness contract).
B, C, H, W = 16, 32, 256, 256
D, KH = 32, 31
TH = TW = 226
NCORES = 8
BPC = B // NCORES  # batches per core
HALO = 30          # extra sigrep rows below a tile (28 group offset + 2 wrap)


def _row_tiles():
    """Output-row tiles (start, nrows); nrows even."""
    r = int(os.environ.get("FFTCONV_R", "38"))
    tiles, i0 = [], 0
    while i0 < TH:
        n = min(r, TH - i0)
        assert n % 2 == 0
        tiles.append((i0, n))
        i0 += n
    return tiles


ROW_TILES = _row_tiles()

# key -> (weight dtype, signal dtype, use 3-D two-row rhs AP of width 229)
# float32r requires a 2-D (flat 512) moving AP; 16-bit dtypes can use the
# narrower 3-D AP (458 streamed columns instead of 512).
_DT_CONFIGS = {
    "f32r": (mybir.dt.float32r, mybir.dt.float32r, False),
    "f16": (mybir.dt.float16, mybir.dt.float16, True),
    "f16flat": (mybir.dt.float16, mybir.dt.float16, False),
    "bf16": (mybir.dt.bfloat16, mybir.dt.bfloat16, True),
    "f32": (mybir.dt.float32, mybir.dt.float32, True),
    # fp8 e4m3: timing probe (precision fails the 2e-2 gate on its own)
    "fp8": (mybir.dt.float8e4, mybir.dt.float8e4, True),
    # e3m4 signal (1-byte moving operand streams at the full 2.4 GHz column
    # rate; 2-byte dtypes cap at ~1.85 GHz) x f16 weights.  Host packs
    # signal*2 -> e3m4 and weight/2 -> f16 so the output scale is exactly 1.
    # Measured rel err vs the fp32 FFT reference: 1.28e-2 (gate: 2e-2).
    "e3mix": (mybir.dt.float16, mybir.dt.float8e3, True),
    # both-e3m4 fallback (if mixed-dtype matmul misbehaves on HW):
    # signal*2 -> e3m4, weight*128 -> e3m4, epilogue rescales by 1/256.
    # rel err 1.81e-2.
    "e3e3": (mybir.dt.float8e3, mybir.dt.float8e3, True),
    # alpha-split: kernel-row groups g<4 stream f16 signal (~1.85 GHz col
    # rate), groups g>=4 stream e4m3 signal (full 2.4 GHz rate; e3m4 does
    # NOT get the fast path, e4m3 does).  Stationary weights all f16, with
    # the g>=4 slices pre-halved so e4m3(2*s) lands on output scale 1.
    # CPU-exact rel err 1.778e-2 (gate 2e-2); HW fp8 numerics match CPU
    # to 4 digits (verified on the e4m3 probe).
    "mix": (mybir.dt.float16, mybir.dt.float16, True),
}
# first g-group index that uses the fp8 signal replica in "mix"
MIX_GSPLIT = int(os.environ.get("FFTCONV_GSPLIT", "4"))
# (signal scale, weight scale, output rescale) applied on the host side.
_DT_SCALES = {
    "e3mix": (2.0, 0.5, 1.0),
    "e3e3": (2.0, 128.0, 1.0 / 256.0),
}
# f16 measured fastest on HW (one LDWEIGHTS per matmul is unavoidable with
# this toolchain; fp16 halves the weight-load and gets FWL).  rel err vs the
# fp32 FFT reference ~2.8e-4; use FFTCONV_DT=f32r for ~1.4e-4 at +15% time.
DT_KEY = os.environ.get("FFTCONV_DT", "f16")
# Timing-only probe: every matmul uses the same stationary operand so
# FFTCONV_LDW_DEDUP can drop ~all Ldweights (output is mathematically
# wrong; use only to measure the zero-LDW PE streaming ceiling).
PROBE_SAME_WT = int(os.environ.get("FFTCONV_PROBE_SAME_WT", "0"))
# Row-pairs per weight-stationary wave (= PSUM banks cycled).  Measured on
# HW: WAVE=1 (bank-stable, weights reloaded per matmul) beats WAVE=8
# (stationary reuse but per-matmul PSUM bank switching stalls the PE).
WAVE = int(os.environ.get("FFTCONV_WAVE", "1"))


def _np_dt(dt_mm):
    return mybir.dt.np(dt_mm)


def build_program(dt_key: str = DT_KEY, repeat: int = 1):
    """Build the SPMD Bass program (one NeuronCore's slice: BPC batches)."""
    wt_dt, dt_mm, use3d = _DT_CONFIGS[dt_key]
    out_rescale = _DT_SCALES.get(dt_key, (1.0, 1.0, 1.0))[2]
    f32 = mybir.dt.float32
    NJ = 229
    # 0 = self-loading matmuls; 1 = explicit ldweights + ldweights=False
    # flags (needs walrus --enable-ldw-opt=false); 2 = ldweights=False flags
    # only (needs --enable-ldw-opt=true, which rejects explicit InstLdweights).
    # Standalone ldweights is rejected for 4-byte dtypes (fp32/fp32r).
    LDW_SHARE = (
        int(os.environ.get("FFTCONV_LDW_SHARE", "0"))
        if wt_dt in (mybir.dt.float16, mybir.dt.bfloat16)
        else 0
    )
    nc = bacc.Bacc(
        "TRN2",
        target_bir_lowering=False,
        debug=False,
        enable_asserts=False,
        num_devices=NCORES,
    )
    is_mix = dt_key == "mix"
    sig_d = nc.dram_tensor("signal", [BPC, C, H, W], dt_mm, kind="ExternalInput")
    sig8_d = (
        nc.dram_tensor("signal8", [BPC, C, H, W], mybir.dt.float8e4, kind="ExternalInput")
        if is_mix
        else None
    )
    wt_d = nc.dram_tensor("wT", [128, 8, 8, 128], wt_dt, kind="ExternalInput")
    bias_d = nc.dram_tensor("bias", [D, 1], f32, kind="ExternalInput")
    out_d = nc.dram_tensor("out", [BPC, D, TH, TW], f32, kind="ExternalOutput")

    SIG_BUFS = int(os.environ.get("FFTCONV_SIG_BUFS", "3"))
    TMP_BUFS = int(os.environ.get("FFTCONV_TMP_BUFS", "4"))
    OUT_BUFS = int(os.environ.get("FFTCONV_OUT_BUFS", "8"))
    with tile.TileContext(nc) as tc:
        with (
            tc.tile_pool(name="const", bufs=1) as const_pool,
            tc.tile_pool(name="sig", bufs=SIG_BUFS) as sig_pool,
            tc.tile_pool(name="psum", bufs=8, space="PSUM") as psum_pool,
            tc.tile_pool(name="tmp", bufs=TMP_BUFS) as tmp_pool,
            tc.tile_pool(name="outb", bufs=OUT_BUFS) as out_pool,
        ):
            wt = const_pool.tile([128, 8, 8, 128], wt_dt)
            nc.sync.dma_start(wt[:, :, :, :], wt_d[:, :, :, :])
            bias_t = const_pool.tile([D, 1], f32)
            nc.sync.dma_start(bias_t[:, :], bias_d[:, :])

            for b in [bb for _ in range(repeat) for bb in range(BPC)]:
                for i0, R in _row_tiles():
                    # Signal rows stored CONTIGUOUSLY at pitch 256 (= W) so a
                    # two-row matmul rhs is one flat 512 span (float32r
                    # requires a 2-D moving AP).  Column overruns wrap to the
                    # next row but only land on zero-weight taps / unread
                    # psum columns.
                    r_tot = R + HALO
                    srep = sig_pool.tile([128, r_tot * W], dt_mm, tag="srep")
                    srep3 = srep[:].rearrange("p (r w) -> p r w", w=W)
                    for u in range(4):
                        rows = min(r_tot, H - (i0 + u))
                        nc.sync.dma_start(
                            srep3[u * 32 : (u + 1) * 32, 0:rows, :],
                            sig_d[b, :, i0 + u : i0 + u + rows, :],
                        )
                        if rows < r_tot:
                            nc.vector.memset(
                                srep3[u * 32 : (u + 1) * 32, rows:r_tot, :].bitcast(
                                    mybir.dt.float32
                                ),
                                0.0,
                            )
                    if is_mix:
                        srep8 = sig_pool.tile(
                            [128, r_tot * W], mybir.dt.float8e4, tag="srep8"
                        )
                        srep83 = srep8[:].rearrange("p (r w) -> p r w", w=W)
                        for u in range(4):
                            rows = min(r_tot, H - (i0 + u))
                            nc.sync.dma_start(
                                srep83[u * 32 : (u + 1) * 32, 0:rows, :],
                                sig8_d[b, :, i0 + u : i0 + u + rows, :],
                            )
                            if rows < r_tot:
                                nc.vector.memset(
                                    srep83[u * 32 : (u + 1) * 32, rows:r_tot, :].bitcast(
                                        mybir.dt.float32
                                    ),
                                    0.0,
                                )
                    all_rps = list(range(R // 2))
                    for w0 in range(0, len(all_rps), WAVE):
                        wave = all_rps[w0 : w0 + WAVE]
                        # Weight-stationary: each (g, vb) lhsT streams all
                        # row-pairs of the wave (distinct PSUM banks) before
                        # the next weight load.
                        ps3s = []
                        for rp in wave:
                            if use3d:
                                ps_t = psum_pool.tile([128, 2, NJ], f32, tag="ps")
                                ps3s.append(ps_t)
                            else:
                                ps_t = psum_pool.tile([128, 2 * W], f32, tag="ps")
                                ps3s.append(ps_t[:].rearrange("p (r w) -> p r w", w=W))
                        # serpentine g order: consecutive row-pairs switch the
                        # moving-operand dtype once instead of twice (mix)
                        g_order = (
                            list(range(8))
                            if (w0 // WAVE) % 2 == 0
                            else list(range(7, -1, -1))
                        )
                        for gi, g in enumerate(g_order):
                            for vb in range(8):
                                # One explicit weight load per (g, vb); the
                                # wave's matmuls reuse the stationary operand
                                # (ldweights=False skips the per-matmul load
                                # walrus would otherwise emit).
                                if LDW_SHARE == 1 and len(wave) > 1:
                                    nc.tensor.ldweights(wt[:, g, vb, :])
                                for j, rp in enumerate(wave):
                                    off = (2 * rp + 4 * g) * W + 4 * vb
                                    ps3 = ps3s[j]
                                    src = srep8 if (is_mix and g >= MIX_GSPLIT) else srep
                                    if use3d:
                                        rhs = src[:, off : off + 2 * W].rearrange(
                                            "p (r w) -> p r w", w=W
                                        )[:, :, 0:NJ]
                                        out_ap = ps3[:, :, :]
                                    else:
                                        rhs = src[:, off : off + 2 * W]
                                        out_ap = ps3.rearrange("p r w -> p (r w)")
                                    wg, wvb = (0, 0) if PROBE_SAME_WT else (g, vb)
                                    mm = nc.tensor.matmul(
                                        out_ap,
                                        lhsT=wt[:, wg, wvb, :],
                                        rhs=rhs,
                                        start=(gi == 0 and vb == 0),
                                        stop=(gi == 7 and vb == 7),
                                    )
                                    if LDW_SHARE and len(wave) > 1 and j > 0:
                                        mm.ins.ldweights = False
                                    elif LDW_SHARE == 1 and len(wave) > 1:
                                        # explicit ldweights above covers it
                                        mm.ins.ldweights = False
                        for j, rp in enumerate(wave):
                            i = i0 + 2 * rp
                            ps3 = ps3s[j]
                            # One PSUM operand per instruction (HW: single DVE
                            # PSUM read port).  ACT folds in the bias.
                            t0 = tmp_pool.tile([D, 2, TW], f32, tag="t0")
                            t1 = tmp_pool.tile([D, 2, TW], f32, tag="t1")
                            t2 = tmp_pool.tile([D, 2, TW], f32, tag="t2")
                            ob = out_pool.tile([D, 2, TW], f32, tag="ob")
                            # Host pre-scales the bias by 1/out_rescale so the
                            # final ACT can apply out_rescale to the whole sum.
                            nc.scalar.activation(
                                t0[:, :, :],
                                ps3[0:32, :, 0:226],
                                mybir.ActivationFunctionType.Identity,
                                bias=bias_t[:, :],
                            )
                            nc.vector.tensor_add(t1[:, :, :], t0[:, :, :], ps3[32:64, :, 1:227])
                            nc.vector.tensor_add(t2[:, :, :], t1[:, :, :], ps3[64:96, :, 2:228])
                            if out_rescale == 1.0:
                                nc.vector.tensor_add(
                                    ob[:, :, :], t2[:, :, :], ps3[96:128, :, 3:229]
                                )
                            else:
                                t3 = tmp_pool.tile([D, 2, TW], f32, tag="t3")
                                nc.vector.tensor_add(
                                    t3[:, :, :], t2[:, :, :], ps3[96:128, :, 3:229]
                                )
                                nc.scalar.activation(
                                    ob[:, :, :],
                                    t3[:, :, :],
                                    mybir.ActivationFunctionType.Identity,
                                    scale=float(out_rescale),
                                )
                            nc.sync.dma_start(out_d[b, :, i : i + 2, :], ob[:, :, :])
    nc.compile()
    # Off by default: only useful with WAVE>1 weight-stationary ordering,
    # which measured slower on HW (PSUM bank cycling).
    if int(os.environ.get("FFTCONV_LDW_DEDUP", "0")):
        bir = _dedupe_ldweights_json(nc.to_json_bytes())
        nc.to_json_bytes = lambda: bir  # instance override; cached bytes
    return nc


def _dedupe_ldweights_json(bir: bytes) -> bytes:
    """Drop PE Ldweights whose stationary operand is already loaded.

    tile_legalize splits every Matmult into Ldweights + Matmult(ldweights
    =false); with weight-stationary waves most loads are redundant reloads
    of the identical operand (measured ~107 ns each, serialized with the
    matmul stream).  Walrus's own dedupe (--enable-ldw-opt) is disabled in
    this toolchain, so do it on the serialized BIR: remove a Ldweights if
    the previous PE array load had the same operands/flags, carrying its
    semaphore waits/updates onto the next PE instruction.
    """
    import json as _json

    j = _json.loads(bir)
    removed = 0
    for fn in j.get("functions", []):
        for blk in fn.get("blocks", []):
            ins_l = blk.get("instructions")
            if not ins_l:
                continue
            out = []
            cur_sig = None
            for inst in ins_l:
                if inst.get("engine") != "PE":
                    out.append(inst)
                    continue
                op = inst.get("opcode")
                if op == "Ldweights":
                    sig = (
                        _json.dumps(inst.get("ins"), sort_keys=True),
                        inst.get("is_transpose"),
                        str(inst.get("perf_mode")),
                        str(inst.get("tile_position")),
                        str(inst.get("tile_size")),
                    )
                    si = inst.get("sync_info") or {}
                    if (
                        sig == cur_sig
                        and not (si.get("on_wait") or si.get("on_update"))
                    ):
                        # bare redundant reload: safe to drop (a Matmult can
                        # hold at most one ISA wait, so loads carrying sync
                        # stay).
                        removed += 1
                        continue
                    cur_sig = sig
                elif op == "Matmult":
                    if inst.get("ldweights") is not False:
                        cur_sig = None  # self-loading matmul replaces stationary
                elif op == "EventSemaphore":
                    pass  # pure semaphore op, array state unaffected
                else:
                    cur_sig = None  # Drain / branch: conservative reset
                out.append(inst)
            blk["instructions"] = out
    if removed:
        sys.stderr.write(f"[kernel] deduped {removed} redundant Ldweights\n")
    return _json.dumps(j).encode()


def pack_weights(weight: np.ndarray, np_dt, scale: float = 1.0) -> np.ndarray:
    """weight [D, C, 31, 31] -> lhsT table [128, 8, 8, 128].

    wT[(u_idx*32 + c), g, vb, (s*32 + d)] = weight[d, c, 4g+u_idx, 4vb+s],
    zero where 4g+u_idx > 30 or 4vb+s > 30.
    """
    w = np.zeros((D, C, 32, 32), np.float32)
    w[:, :, :KH, :KH] = weight.astype(np.float32) * scale
    # -> [u_idx, c, g, vb, s, d]
    wt = w.reshape(D, C, 8, 4, 8, 4).transpose(3, 1, 2, 4, 5, 0)
    wt = wt.reshape(4 * C, 8, 8, 4 * D)
    return np.ascontiguousarray(wt.astype(np_dt))


def pack_weights_mix(weight: np.ndarray) -> np.ndarray:
    """f16 lhsT table; g>=MIX_GSPLIT slices halved (fp8 signal is 2*s)."""
    wt = pack_weights(weight, np.float32)
    wt[:, MIX_GSPLIT:, :, :] *= 0.5
    return np.ascontiguousarray(wt.astype(np.float16))


_PROGRAM_CACHE: dict[str, object] = {}


def _get_program(dt_key: str):
    key = (dt_key, WAVE)
    prog = _PROGRAM_CACHE.get(key)
    if prog is None:
        prog = build_program(dt_key)
        _PROGRAM_CACHE[key] = prog
    return prog


def make_in_maps(signal, weight, bias, dt_key: str = DT_KEY):
    wt_dt, sig_dt, _ = _DT_CONFIGS[dt_key]
    sig_scale, wt_scale, out_rescale = _DT_SCALES.get(dt_key, (1.0, 1.0, 1.0))
    sig8 = None
    if dt_key == "mix":
        wT = pack_weights_mix(np.asarray(weight))
        sig = np.asarray(signal, dtype=np.float32)
        sig8 = np.clip(sig * 2.0, -448.0, 448.0).astype(_np_dt(mybir.dt.float8e4))
        sig = sig.astype(np.float16)
    else:
        wT = pack_weights(np.asarray(weight), _np_dt(wt_dt), wt_scale)
        sig = np.asarray(signal, dtype=np.float32)
        if sig_scale != 1.0:
            # clip into the e3m4 finite range so astype can't produce inf
            sig = np.clip(sig * sig_scale, -15.5, 15.5)
        sig = sig.astype(_np_dt(sig_dt), copy=False)
    b2 = np.ascontiguousarray(
        np.asarray(bias, np.float32).reshape(D, 1) / out_rescale
    )
    maps = [
        {
            "signal": np.ascontiguousarray(sig[c * BPC : (c + 1) * BPC]),
            "wT": wT,
            "bias": b2,
        }
        for c in range(NCORES)
    ]
    if sig8 is not None:
        for c in range(NCORES):
            maps[c]["signal8"] = np.ascontiguousarray(sig8[c * BPC : (c + 1) * BPC])
    return maps


class _Executor:
    """Cached jitted shard_map executor (re-jitting per call costs ~7 s).

    Outputs are fully written by the kernel each run, so the previous
    call's output buffers are donated as the next call's NEFF output
    operands (no fresh zero upload per call).
    """

    def __init__(self, nc):
        import jax
        from concourse.bass2jax import (
            _bass_exec_p,
            install_neuronx_cc_hook,
            partition_id_tensor,
        )
        from jax.sharding import Mesh, NamedSharding, PartitionSpec

        try:
            from jax.experimental.shard_map import shard_map
        except ImportError:
            from jax import shard_map

        install_neuronx_cc_hook()
        self.jax = jax
        part_name = nc.partition_id_tensor.name if nc.partition_id_tensor else None
        in_names, out_names, out_avals = [], [], []
        for alloc in nc.m.functions[0].allocations:
            if not isinstance(alloc, mybir.MemoryLocationSet):
                continue
            name = alloc.memorylocations[0].name
            if alloc.kind == "ExternalInput":
                if name != part_name:
                    in_names.append(name)
            elif alloc.kind == "ExternalOutput":
                out_names.append(name)
                out_avals.append(
                    jax.core.ShapedArray(
                        tuple(alloc.tensor_shape), mybir.dt.np(alloc.dtype)
                    )
                )
        self.in_names, self.out_names, self.out_avals = in_names, out_names, out_avals
        n_params = len(in_names)
        all_in = list(in_names) + list(out_names)
        if part_name is not None:
            all_in.append(part_name)

        def _body(*args):
            operands = list(args)
            if part_name is not None:
                operands.append(partition_id_tensor())
            return tuple(
                _bass_exec_p.bind(
                    *operands,
                    out_avals=tuple(out_avals),
                    in_names=tuple(all_in),
                    out_names=tuple(out_names),
                    lowering_input_output_aliases=(),
                    sim_require_finite=True,
                    sim_require_nnan=True,
                    nc=nc,
                )
            )

        devices = jax.devices()[:NCORES]
        mesh = Mesh(np.asarray(devices), ("core",))
        n_outs = len(out_names)
        self.fn = jax.jit(
            shard_map(
                _body,
                mesh=mesh,
                in_specs=(PartitionSpec("core"),) * (n_params + n_outs),
                out_specs=(PartitionSpec("core"),) * n_outs,
                check_rep=False,
            ),
            donate_argnums=tuple(range(n_params, n_params + n_outs)),
        )
        self.in_sharding = NamedSharding(mesh, PartitionSpec("core"))
        self.prev_outs = None

    def run(self, in_maps):
        jax = self.jax
        concat_in = [
            np.concatenate([np.asarray(m[n]) for m in in_maps], axis=0)
            for n in self.in_names
        ]
        dev_in = jax.device_put(concat_in, [self.in_sharding] * len(concat_in))
        outs = self.prev_outs
        if outs is None:
            outs = [
                np.zeros((NCORES * a.shape[0], *a.shape[1:]), a.dtype)
                for a in self.out_avals
            ]
        outs = self.fn(*dev_in, *outs)
        jax.block_until_ready(outs)
        host = {n: np.asarray(o) for n, o in zip(self.out_names, outs)}
        self.prev_outs = list(outs)
        return host


_EXECUTOR_CACHE: dict = {}


def _get_executor():
    key = (DT_KEY, WAVE)
    ex = _EXECUTOR_CACHE.get(key)
    if ex is None:
        ex = _Executor(_get_program(DT_KEY))
        _EXECUTOR_CACHE[key] = ex
    return ex


def kernel(signal, weight, bias):
    in_maps = make_in_maps(signal, weight, bias, DT_KEY)
    try:
        host = _get_executor().run(in_maps)
        out_full = host["out"]
    except Exception:
        # Fallback: the stock (slower, re-jitting) execution path.
        nc = _get_program(DT_KEY)
        res = run_bass_kernel_spmd(nc, in_maps, list(range(NCORES)))
        out_full = np.concatenate(
            [res.results[c]["out"] for c in range(NCORES)], axis=0
        )
    out = out_full.reshape(B, D, TH, TW)
    return np.ascontiguousarray(out.astype(np.float32, copy=False))



# revision 21
# speedup vs baseline: 1.2981x; 1.2981x over previous
"""FFT-Conv2d as direct valid cross-correlation on Trainium2 (Bass/Tile).

Math: the reference's rfft2/einsum/irfft2 pipeline is exactly a *valid*
2-D cross-correlation plus bias:

    out[b, d, i, j] = sum_{c,u,v} signal[b, c, i+u, j+v] * weight[d, c, u, v]
                      + bias[d]

with signal [16, 32, 256, 256], weight [32, 32, 31, 31] -> out [16, 32, 226, 226].

Device strategy (data-parallel, 2 batch images per NeuronCore x 8 cores):
  - Contraction dim (PE partition axis, K=128) packs 4 kernel rows x 32
    input channels.  The signal is replicated into SBUF 4x with row shifts
    of 0..3 so that one SBUF row slice provides all 128 contraction rows.
  - Output dim (PE partition axis of PSUM, M=128) packs 4 kernel-column
    subshifts s=0..3 x 32 output channels.  A column block vb covers
    kernel columns 4*vb+s; the s-shift is resolved after accumulation by
    a shifted 4-way add across PSUM partition blocks.
  - Per output-row-pair: 8 row-groups x 8 col-blocks = 64 matmuls of
    [128 x 128] @ [128 x (2*229)] accumulated in one PSUM bank, then a
    3-op vector epilogue (2 shifted adds + fused add+bias) and a DMA out.

Kernel weights/columns beyond 31 are zero-padded on the host; the signal
is zero-padded by one row/column in SBUF so the padded taps multiply
zeros (never uninitialized memory).

Dtype strategy (FFTCONV_DT=f16 default).  HW findings from this tuning
session (all at N=458 free-dim, 14464 matmuls/core):

  - The PE moving-operand stream rate is dtype-dependent: e4m3 runs the
    full 2.4 GHz column rate (198 ns/MM), f16 ~1.85 GHz (247 ns), e3m4
    ~1.88 GHz, bf16 ~1.55 GHz (295 ns), f32r ~1.61 GHz.
  - LDWEIGHTS is fully hidden under f16 matmuls (removing 98% of the
    Ldweights via BIR dedup changed nothing), and PSUM-bank cycling
    across waves is free (WAVE=4 + dedup == WAVE=1).
  - e4m3 end-to-end measures 3.07 ms but rel err 4.1e-2 > the 2e-2
    gate.  e3m4 passes (1.28e-2 mixed f16 weights / 1.81e-2 pure) but
    streams at the slow 16-bit rate, so it is not faster than f16.
    HW fp8 numerics match the CPU quantization model to 4 digits
    (subnormals honored), so these error numbers are exact.
  - An alpha-split (kernel-row groups g>=4 streaming e4m3(2*signal)
    with halved f16 stationary weights, rel err 1.78e-2) does NOT
    recover the e4m3 rate: any mixing of moving dtypes in the stream
    forfeits the fast path (3.64 ms interleaved, 3.84 ms in 128-MM
    same-dtype bursts vs 3.57 ms pure f16).

Hence f16 everywhere: 3.57 ms measured, rel err 2.8e-4, which is ~97%
of this scheme's pure-streaming envelope (no-LDW probe: 3.565 ms).
The mix/e3mix/e3e3/fp8 paths are kept selectable via FFTCONV_DT for
reference.
"""

import os
import sys

import numpy as np

for _p in ("/opt/trn_rl_repo",):
    if _p not in sys.path and os.path.isdir(_p):
        sys.path.insert(0, _p)

import concourse.bacc as bacc
import concourse.mybir as mybir
import concourse.tile as tile
from concourse.bass_utils import run_bass_kernel_spmd

# Problem constants (hardcoded per harness contract).
B, C, H, W = 16, 32, 256, 256
D, KH = 32, 31
TH = TW = 226
NCORES = 8
BPC = B // NCORES  # batches per core
HALO = 30          # extra sigrep rows below a tile (28 group offset + 2 wrap)


def _row_tiles():
    """Output-row tiles (start, nrows); nrows even."""
    r = int(os.environ.get("FFTCONV_R", "38"))
    tiles, i0 = [], 0
    while i0 < TH:
        n = min(r, TH - i0)
        assert n % 2 == 0
        tiles.append((i0, n))
        i0 += n
    return tiles


ROW_TILES = _row_tiles()

# key -> (weight dtype, signal dtype, use 3-D two-row rhs AP of width 229)
# float32r requires a 2-D (flat 512) moving AP; 16-bit dtypes can use the
# narrower 3-D AP (458 streamed columns instead of 512).
_DT_CONFIGS = {
    "f32r": (mybir.dt.float32r, mybir.dt.float32r, False),
    "f16": (mybir.dt.float16, mybir.dt.float16, True),
    "f16flat": (mybir.dt.float16, mybir.dt.float16, False),
    "bf16": (mybir.dt.bfloat16, mybir.dt.bfloat16, True),
    "f32": (mybir.dt.float32, mybir.dt.float32, True),
    # fp8 e4m3: timing probe (precision fails the 2e-2 gate on its own)
    "fp8": (mybir.dt.float8e4, mybir.dt.float8e4, True),
    # e3m4 signal (1-byte moving operand streams at the full 2.4 GHz column
    # rate; 2-byte dtypes cap at ~1.85 GHz) x f16 weights.  Host packs
    # signal*2 -> e3m4 and weight/2 -> f16 so the output scale is exactly 1.
    # Measured rel err vs the fp32 FFT reference: 1.28e-2 (gate: 2e-2).
    "e3mix": (mybir.dt.float16, mybir.dt.float8e3, True),
    # both-e3m4 fallback (if mixed-dtype matmul misbehaves on HW):
    # signal*2 -> e3m4, weight*128 -> e3m4, epilogue rescales by 1/256.
    # rel err 1.81e-2.
    "e3e3": (mybir.dt.float8e3, mybir.dt.float8e3, True),
    # alpha-split: kernel-row groups g<4 stream f16 signal (~1.85 GHz col
    # rate), groups g>=4 stream e4m3 signal (full 2.4 GHz rate; e3m4 does
    # NOT get the fast path, e4m3 does).  Stationary weights all f16, with
    # the g>=4 slices pre-halved so e4m3(2*s) lands on output scale 1.
    # CPU-exact rel err 1.778e-2 (gate 2e-2); HW fp8 numerics match CPU
    # to 4 digits (verified on the e4m3 probe).
    "mix": (mybir.dt.float16, mybir.dt.float16, True),
}
# first g-group index that uses the fp8 signal replica in "mix"
MIX_GSPLIT = int(os.environ.get("FFTCONV_GSPLIT", "4"))
# (signal scale, weight scale, output rescale) applied on the host side.
_DT_SCALES = {
    "e3mix": (2.0, 0.5, 1.0),
    "e3e3": (2.0, 128.0, 1.0 / 256.0),
}
# f16 measured fastest on HW (one LDWEIGHTS per matmul is unavoidable with
# this toolchain; fp16 halves the weight-load and gets FWL).  rel err vs the
# fp32 FFT reference ~2.8e-4; use FFTCONV_DT=f32r for ~1.4e-4 at +15% time.
DT_KEY = os.environ.get("FFTCONV_DT", "f16")
# Timing-only probe: every matmul uses the same stationary operand so
# FFTCONV_LDW_DEDUP can drop ~all Ldweights (output is mathematically
# wrong; use only to measure the zero-LDW PE streaming ceiling).
PROBE_SAME_WT = int(os.environ.get("FFTCONV_PROBE_SAME_WT", "0"))
# Row-pairs per weight-stationary wave (= PSUM banks cycled).  Measured on
# HW: WAVE=1 (bank-stable, weights reloaded per matmul) beats WAVE=8
# (stationary reuse but per-matmul PSUM bank switching stalls the PE).
WAVE = int(os.environ.get("FFTCONV_WAVE", "1"))


def _np_dt(dt_mm):
    return mybir.dt.np(dt_mm)


def build_program(dt_key: str = DT_KEY, repeat: int = 1):
    """Build the SPMD Bass program (one NeuronCore's slice: BPC batches)."""
    wt_dt, dt_mm, use3d = _DT_CONFIGS[dt_key]
    out_rescale = _DT_SCALES.get(dt_key, (1.0, 1.0, 1.0))[2]
    f32 = mybir.dt.float32
    NJ = 229
    # 0 = self-loading matmuls; 1 = explicit ldweights + ldweights=False
    # flags (needs walrus --enable-ldw-opt=false); 2 = ldweights=False flags
    # only (needs --enable-ldw-opt=true, which rejects explicit InstLdweights).
    # Standalone ldweights is rejected for 4-byte dtypes (fp32/fp32r).
    LDW_SHARE = (
        int(os.environ.get("FFTCONV_LDW_SHARE", "0"))
        if wt_dt in (mybir.dt.float16, mybir.dt.bfloat16)
        else 0
    )
    nc = bacc.Bacc(
        "TRN2",
        target_bir_lowering=False,
        debug=False,
        enable_asserts=False,
        num_devices=NCORES,
    )
    is_mix = dt_key == "mix"
    sig_d = nc.dram_tensor("signal", [BPC, C, H, W], dt_mm, kind="ExternalInput")
    sig8_d = (
        nc.dram_tensor("signal8", [BPC, C, H, W], mybir.dt.float8e4, kind="ExternalInput")
        if is_mix
        else None
    )
    wt_d = nc.dram_tensor("wT", [128, 8, 8, 128], wt_dt, kind="ExternalInput")
    bias_d = nc.dram_tensor("bias", [D, 1], f32, kind="ExternalInput")
    out_d = nc.dram_tensor("out", [BPC, D, TH, TW], f32, kind="ExternalOutput")

    SIG_BUFS = int(os.environ.get("FFTCONV_SIG_BUFS", "3"))
    TMP_BUFS = int(os.environ.get("FFTCONV_TMP_BUFS", "4"))
    OUT_BUFS = int(os.environ.get("FFTCONV_OUT_BUFS", "8"))
    with tile.TileContext(nc) as tc:
        with (
            tc.tile_pool(name="const", bufs=1) as const_pool,
            tc.tile_pool(name="sig", bufs=SIG_BUFS) as sig_pool,
            tc.tile_pool(name="psum", bufs=8, space="PSUM") as psum_pool,
            tc.tile_pool(name="tmp", bufs=TMP_BUFS) as tmp_pool,
            tc.tile_pool(name="outb", bufs=OUT_BUFS) as out_pool,
        ):
            wt = const_pool.tile([128, 8, 8, 128], wt_dt)
            nc.sync.dma_start(wt[:, :, :, :], wt_d[:, :, :, :])
            bias_t = const_pool.tile([D, 1], f32)
            nc.sync.dma_start(bias_t[:, :], bias_d[:, :])

            for b in [bb for _ in range(repeat) for bb in range(BPC)]:
                for i0, R in _row_tiles():
                    # Signal rows stored CONTIGUOUSLY at pitch 256 (= W) so a
                    # two-row matmul rhs is one flat 512 span (float32r
                    # requires a 2-D moving AP).  Column overruns wrap to the
                    # next row but only land on zero-weight taps / unread
                    # psum columns.
                    r_tot = R + HALO
                    srep = sig_pool.tile([128, r_tot * W], dt_mm, tag="srep")
                    srep3 = srep[:].rearrange("p (r w) -> p r w", w=W)
                    for u in range(4):
                        rows = min(r_tot, H - (i0 + u))
                        nc.sync.dma_start(
                            srep3[u * 32 : (u + 1) * 32, 0:rows, :],
                            sig_d[b, :, i0 + u : i0 + u + rows, :],
                        )
                        if rows < r_tot:
                            nc.vector.memset(
                                srep3[u * 32 : (u + 1) * 32, rows:r_tot, :].bitcast(
                                    mybir.dt.float32
                                ),
                                0.0,
                            )
                    if is_mix:
                        srep8 = sig_pool.tile(
                            [128, r_tot * W], mybir.dt.float8e4, tag="srep8"
                        )
                        srep83 = srep8[:].rearrange("p (r w) -> p r w", w=W)
                        for u in range(4):
                            rows = min(r_tot, H - (i0 + u))
                            nc.sync.dma_start(
                                srep83[u * 32 : (u + 1) * 32, 0:rows, :],
                                sig8_d[b, :, i0 + u : i0 + u + rows, :],
                            )
                            if rows < r_tot:
                                nc.vector.memset(
                                    srep83[u * 32 : (u + 1) * 32, rows:r_tot, :].bitcast(
                                        mybir.dt.float32
                                    ),
                                    0.0,
                                )
                    all_rps = list(range(R // 2))
                    for w0 in range(0, len(all_rps), WAVE):
                        wave = all_rps[w0 : w0 + WAVE]
                        # Weight-stationary: each (g, vb) lhsT streams all
                        # row-pairs of the wave (distinct PSUM banks) before
                        # the next weight load.
                        ps3s = []
                        for rp in wave:
                            if use3d:
                                ps_t = psum_pool.tile([128, 2, NJ], f32, tag="ps")
                                ps3s.append(ps_t)
                            else:
                                ps_t = psum_pool.tile([128, 2 * W], f32, tag="ps")
                                ps3s.append(ps_t[:].rearrange("p (r w) -> p r w", w=W))
                        # serpentine g order: consecutive row-pairs switch the
                        # moving-operand dtype once instead of twice (mix)
                        g_order = (
                            list(range(8))
                            if (w0 // WAVE) % 2 == 0
                            else list(range(7, -1, -1))
                        )
                        for gi, g in enumerate(g_order):
                            for vb in range(8):
                                # One explicit weight load per (g, vb); the
                                # wave's matmuls reuse the stationary operand
                                # (ldweights=False skips the per-matmul load
                                # walrus would otherwise emit).
                                if LDW_SHARE == 1 and len(wave) > 1:
                                    nc.tensor.ldweights(wt[:, g, vb, :])
                                for j, rp in enumerate(wave):
                                    off = (2 * rp + 4 * g) * W + 4 * vb
                                    ps3 = ps3s[j]
                                    src = srep8 if (is_mix and g >= MIX_GSPLIT) else srep
                                    if use3d:
                                        rhs = src[:, off : off + 2 * W].rearrange(
                                            "p (r w) -> p r w", w=W
                                        )[:, :, 0:NJ]
                                        out_ap = ps3[:, :, :]
                                    else:
                                        rhs = src[:, off : off + 2 * W]
                                        out_ap = ps3.rearrange("p r w -> p (r w)")
                                    wg, wvb = (0, 0) if PROBE_SAME_WT else (g, vb)
                                    mm = nc.tensor.matmul(
                                        out_ap,
                                        lhsT=wt[:, wg, wvb, :],
                                        rhs=rhs,
                                        start=(gi == 0 and vb == 0),
                                        stop=(gi == 7 and vb == 7),
                                    )
                                    if LDW_SHARE and len(wave) > 1 and j > 0:
                                        mm.ins.ldweights = False
                                    elif LDW_SHARE == 1 and len(wave) > 1:
                                        # explicit ldweights above covers it
                                        mm.ins.ldweights = False
                        for j, rp in enumerate(wave):
                            i = i0 + 2 * rp
                            ps3 = ps3s[j]
                            # One PSUM operand per instruction (HW: single DVE
                            # PSUM read port).  ACT folds in the bias.
                            t0 = tmp_pool.tile([D, 2, TW], f32, tag="t0")
                            t1 = tmp_pool.tile([D, 2, TW], f32, tag="t1")
                            t2 = tmp_pool.tile([D, 2, TW], f32, tag="t2")
                            ob = out_pool.tile([D, 2, TW], f32, tag="ob")
                            # Host pre-scales the bias by 1/out_rescale so the
                            # final ACT can apply out_rescale to the whole sum.
                            nc.scalar.activation(
                                t0[:, :, :],
                                ps3[0:32, :, 0:226],
                                mybir.ActivationFunctionType.Identity,
                                bias=bias_t[:, :],
                            )
                            nc.vector.tensor_add(t1[:, :, :], t0[:, :, :], ps3[32:64, :, 1:227])
                            nc.vector.tensor_add(t2[:, :, :], t1[:, :, :], ps3[64:96, :, 2:228])
                            if out_rescale == 1.0:
                                nc.vector.tensor_add(
                                    ob[:, :, :], t2[:, :, :], ps3[96:128, :, 3:229]
                                )
                            else:
                                t3 = tmp_pool.tile([D, 2, TW], f32, tag="t3")
                                nc.vector.tensor_add(
                                    t3[:, :, :], t2[:, :, :], ps3[96:128, :, 3:229]
                                )
                                nc.scalar.activation(
                                    ob[:, :, :],
                                    t3[:, :, :],
                                    mybir.ActivationFunctionType.Identity,
                                    scale=float(out_rescale),
                                )
                            nc.sync.dma_start(out_d[b, :, i : i + 2, :], ob[:, :, :])
    nc.compile()
    # Off by default: only useful with WAVE>1 weight-stationary ordering,
    # which measured slower on HW (PSUM bank cycling).
    if int(os.environ.get("FFTCONV_LDW_DEDUP", "0")):
        bir = _dedupe_ldweights_json(nc.to_json_bytes())
        nc.to_json_bytes = lambda: bir  # instance override; cached bytes
    return nc


def _dedupe_ldweights_json(bir: bytes) -> bytes:
    """Drop PE Ldweights whose stationary operand is already loaded.

    tile_legalize splits every Matmult into Ldweights + Matmult(ldweights
    =false); with weight-stationary waves most loads are redundant reloads
    of the identical operand (measured ~107 ns each, serialized with the
    matmul stream).  Walrus's own dedupe (--enable-ldw-opt) is disabled in
    this toolchain, so do it on the serialized BIR: remove a Ldweights if
    the previous PE array load had the same operands/flags, carrying its
    semaphore waits/updates onto the next PE instruction.
    """
    import json as _json

    j = _json.loads(bir)
    removed = 0
    for fn in j.get("functions", []):
        for blk in fn.get("blocks", []):
            ins_l = blk.get("instructions")
            if not ins_l:
                continue
            out = []
            cur_sig = None
            for inst in ins_l:
                if inst.get("engine") != "PE":
                    out.append(inst)
                    continue
                op = inst.get("opcode")
                if op == "Ldweights":
                    sig = (
                        _json.dumps(inst.get("ins"), sort_keys=True),
                        inst.get("is_transpose"),
                        str(inst.get("perf_mode")),
                        str(inst.get("tile_position")),
                        str(inst.get("tile_size")),
                    )
                    si = inst.get("sync_info") or {}
                    if (
                        sig == cur_sig
                        and not (si.get("on_wait") or si.get("on_update"))
                    ):
                        # bare redundant reload: safe to drop (a Matmult can
                        # hold at most one ISA wait, so loads carrying sync
                        # stay).
                        removed += 1
                        continue
                    cur_sig = sig
                elif op == "Matmult":
                    if inst.get("ldweights") is not False:
                        cur_sig = None  # self-loading matmul replaces stationary
                elif op == "EventSemaphore":
                    pass  # pure semaphore op, array state unaffected
                else:
                    cur_sig = None  # Drain / branch: conservative reset
                out.append(inst)
            blk["instructions"] = out
    if removed:
        sys.stderr.write(f"[kernel] deduped {removed} redundant Ldweights\n")
    return _json.dumps(j).encode()


def pack_weights(weight: np.ndarray, np_dt, scale: float = 1.0) -> np.ndarray:
    """weight [D, C, 31, 31] -> lhsT table [128, 8, 8, 128].

    wT[(u_idx*32 + c), g, vb, (s*32 + d)] = weight[d, c, 4g+u_idx, 4vb+s],
    zero where 4g+u_idx > 30 or 4vb+s > 30.
    """
    w = np.zeros((D, C, 32, 32), np.float32)
    w[:, :, :KH, :KH] = weight.astype(np.float32) * scale
    # -> [u_idx, c, g, vb, s, d]
    wt = w.reshape(D, C, 8, 4, 8, 4).transpose(3, 1, 2, 4, 5, 0)
    wt = wt.reshape(4 * C, 8, 8, 4 * D)
    return np.ascontiguousarray(wt.astype(np_dt))


def pack_weights_mix(weight: np.ndarray) -> np.ndarray:
    """f16 lhsT table; g>=MIX_GSPLIT slices halved (fp8 signal is 2*s)."""
    wt = pack_weights(weight, np.float32)
    wt[:, MIX_GSPLIT:, :, :] *= 0.5
    return np.ascontiguousarray(wt.astype(np.float16))


_PROGRAM_CACHE: dict[str, object] = {}


def _get_program(dt_key: str):
    key = (dt_key, WAVE)
    prog = _PROGRAM_CACHE.get(key)
    if prog is None:
        prog = build_program(dt_key)
        _PROGRAM_CACHE[key] = prog
    return prog


def make_in_maps(signal, weight, bias, dt_key: str = DT_KEY):
    wt_dt, sig_dt, _ = _DT_CONFIGS[dt_key]
    sig_scale, wt_scale, out_rescale = _DT_SCALES.get(dt_key, (1.0, 1.0, 1.0))
    sig8 = None
    if dt_key == "mix":
        wT = pack_weights_mix(np.asarray(weight))
        sig = np.asarray(signal, dtype=np.float32)
        sig8 = np.clip(sig * 2.0, -448.0, 448.0).astype(_np_dt(mybir.dt.float8e4))
        sig = sig.astype(np.float16)
    else:
        wT = pack_weights(np.asarray(weight), _np_dt(wt_dt), wt_scale)
        sig = np.asarray(signal, dtype=np.float32)
        if sig_scale != 1.0:
            # clip into the e3m4 finite range so astype can't produce inf
            sig = np.clip(sig * sig_scale, -15.5, 15.5)
        sig = sig.astype(_np_dt(sig_dt), copy=False)
    b2 = np.ascontiguousarray(
        np.asarray(bias, np.float32).reshape(D, 1) / out_rescale
    )
    maps = [
        {
            "signal": np.ascontiguousarray(sig[c * BPC : (c + 1) * BPC]),
            "wT": wT,
            "bias": b2,
        }
        for c in range(NCORES)
    ]
    if sig8 is not None:
        for c in range(NCORES):
            maps[c]["signal8"] = np.ascontiguousarray(sig8[c * BPC : (c + 1) * BPC])
    return maps


class _Executor:
    """Cached jitted shard_map executor (re-jitting per call costs ~7 s).

    Outputs are fully written by the kernel each run, so the previous
    call's output buffers are donated as the next call's NEFF output
    operands (no fresh zero upload per call).
    """

    def __init__(self, nc):
        import jax
        from concourse.bass2jax import (
            _bass_exec_p,
            install_neuronx_cc_hook,
            partition_id_tensor,
        )
        from jax.sharding import Mesh, NamedSharding, PartitionSpec

        try:
            from jax.experimental.shard_map import shard_map
        except ImportError:
            from jax import shard_map

        install_neuronx_cc_hook()
        self.jax = jax
        part_name = nc.partition_id_tensor.name if nc.partition_id_tensor else None
        in_names, out_names, out_avals = [], [], []
        for alloc in nc.m.functions[0].allocations:
            if not isinstance(alloc, mybir.MemoryLocationSet):
                continue
            name = alloc.memorylocations[0].name
            if alloc.kind == "ExternalInput":
                if name != part_name:
                    in_names.append(name)
            elif alloc.kind == "ExternalOutput":
                out_names.append(name)
                out_avals.append(
                    jax.core.ShapedArray(
                        tuple(alloc.tensor_shape), mybir.dt.np(alloc.dtype)
                    )
                )
        self.in_names, self.out_names, self.out_avals = in_names, out_names, out_avals
        n_params = len(in_names)
        all_in = list(in_names) + list(out_names)
        if part_name is not None:
            all_in.append(part_name)

        def _body(*args):
            operands = list(args)
            if part_name is not None:
                operands.append(partition_id_tensor())
            return tuple(
                _bass_exec_p.bind(
                    *operands,
                    out_avals=tuple(out_avals),
                    in_names=tuple(all_in),
                    out_names=tuple(out_names),
                    lowering_input_output_aliases=(),
                    sim_require_finite=True,
                    sim_require_nnan=True,
                    nc=nc,
                )
            )

        devices = jax.devices()[:NCORES]
        mesh = Mesh(np.asarray(devices), ("core",))
        n_outs = len(out_names)
        self.fn = jax.jit(
            shard_map(
                _body,
                mesh=mesh,
                in_specs=(PartitionSpec("core"),) * (n_params + n_outs),
                out_specs=(PartitionSpec("core"),) * n_outs,
                check_rep=False,
            ),
            donate_argnums=tuple(range(n_params, n_params + n_outs)),
        )
        self.in_sharding = NamedSharding(mesh, PartitionSpec("core"))
        self.prev_outs = None

    def run(self, in_maps):
        jax = self.jax
        concat_in = [
            np.concatenate([np.asarray(m[n]) for m in in_maps], axis=0)
            for n in self.in_names
        ]
        dev_in = jax.device_put(concat_in, [self.in_sharding] * len(concat_in))
        outs = self.prev_outs
        if outs is None:
            outs = [
                np.zeros((NCORES * a.shape[0], *a.shape[1:]), a.dtype)
                for a in self.out_avals
            ]
        outs = self.fn(*dev_in, *outs)
        jax.block_until_ready(outs)
        host = {n: np.asarray(o) for n, o in zip(self.out_names, outs)}
        self.prev_outs = list(outs)
        return host


_EXECUTOR_CACHE: dict = {}


def _get_executor():
    key = (DT_KEY, WAVE)
    ex = _EXECUTOR_CACHE.get(key)
    if ex is None:
        ex = _Executor(_get_program(DT_KEY))
        _EXECUTOR_CACHE[key] = ex
    return ex


def kernel(signal, weight, bias):
    in_maps = make_in_maps(signal, weight, bias, DT_KEY)
    try:
        host = _get_executor().run(in_maps)
        out_full = host["out"]
    except Exception:
        # Fallback: the stock (slower, re-jitting) execution path.
        nc = _get_program(DT_KEY)
        res = run_bass_kernel_spmd(nc, in_maps, list(range(NCORES)))
        out_full = np.concatenate(
            [res.results[c]["out"] for c in range(NCORES)], axis=0
        )
    out = out_full.reshape(B, D, TH, TW)
    return np.ascontiguousarray(out.astype(np.float32, copy=False))

